# revision 53
# baseline (speedup 1.0000x reference)
"""Trainium2 Bass kernel for the SCON linear-SDE particle scan (bf16 datapath).

Reference computation: x_{t+1} = (I + DT*W_{t+1}) x_t + DT*b_{t+1} + ds*eps_t
over 10000 steps for B=512 particles with a 3-dim state, observed every 50
steps through a [4,3] projection -> loc_y [512, 201, 4].

The transition matrices depend only on theta (14 scalars), so the whole scan
is a linear map of (x0, eps).  On the host (float64) we precompute hierarchical
propagator weights that turn the scan into three levels of PE matmuls over the
noise tensor:

  level A: chunks of 10 steps   U10[c]  = sum_t  S10[c,t] ds eps_t   (1000)
  level B: windows of 50 steps  U50[w]  = sum_g  S50[w,g] U10[5w+g]  (200)
  level C: obs propagation + projection; the x0/deterministic affine part is
           computed on host (float64) and added during PSUM evacuation.

The noise datapath is bf16 (error ~4e-3 vs the f32 reference: the output is
a sum of thousands of independently-rounded noise terms), which halves DMA
bytes and PE matmul passes vs f32 (fp32 matmuls run the array twice).  The
kernel is DMA-bound (~400 GB/s sustained on one HWDGE ring): eps streams in 6
slices aligned with the level-A matmul groups, PE/DVE/ACT work is emitted in
data-arrival order to avoid head-of-line blocking in the engine FIFOs, and
the last 8 windows bypass levels A/B via host-combined weights (wd) so only
a short matmul chain trails the final 154 KB slice.  PSUM tiles are zeroed
by PE zero-weight matmuls during the startup window (garbage PSUM bits can
be NaN; 0*NaN would poison downstream accumulations).

B is sharded 64 particles per core across 8 cores (pure data parallel).
"""

import numpy as np
import ml_dtypes

BF16 = ml_dtypes.bfloat16

# ---------------------------------------------------------------- constants
T_TOT = 1000.0
DT = 0.1
N = 10001
TEMP_REF = 283.0
TEMP_RISE = 5.0
GAS_R = 0.008314
NSTEP = N - 1            # 10000
B = 512
NCORE = 8
BC = B // NCORE          # 64 particles per core

L1 = 10                  # level-A chunk length (steps)
NC1 = NSTEP // L1        # 1000 chunks
CPW = 5                  # chunks per window
NW = NC1 // CPW          # 200 windows
NOBS = NW + 1            # 201 observations
OBS_EVERY = 50

SUPER = 4                # chunks per level-A matmul
NMM_A = NC1 // SUPER     # 250 level-A matmuls
KE = 128                 # eps rows per level-A matmul (4 chunks x 32; rows
                         # 30,31 of each block are zero pad -- engine ops
                         # must start at 32-aligned partitions)
NSUP_COL = 250           # eps columns groups (one per matmul)
NTILE_A = (NMM_A + 3) // 4   # 63 u10 column tiles (4 matmuls/tile, last has 2)

WPS = 10                 # windows per level-B slot (30 rows of 32)
NSLOT_B = NW // WPS      # 20 slots
NTILE_B = NSLOT_B // 4   # 5 u50 tiles
TAUS_PER_SLOT = 4        # u10 tiles touched per level-B slot

NOUT = 4 * NOBS          # 804
NH = NOUT // 2           # 402  (psum free-dim per matmul)

# level-A big-groups (sgA ranges) -> one PSUM tile + one cast each.  eps DMA
# slices align with big pairs (1 MB keeps DMA at line rate), so level-A work
# streams with the data.  The last 8 windows (supergroups 240-249, the final
# 154 KB slice) bypass levels A/B entirely: host-precomputed combined
# weights (wd) map their eps directly to output cols [772:804], so the
# post-DMA tail is one short matmul chain instead of A->cast->B->copy->C.
BIGS = [(0, 8), (8, 16), (16, 24), (24, 32), (32, 40),
        (40, 48), (48, 56), (56, 60)]
SLICES = [64, 64, 64, 32, 16, 10]
SOFF = [0, 64, 128, 192, 224, 240, 250]
NSD = 10                 # direct-path supergroups (s 240-249)
NOD = 32                 # direct-path output cols (obs 193-200)

_program_cache = None
_last_results = None     # BassKernelResults of the most recent run (for test.py)

# ------------------------------------------------------------- host math
def _forcings():
    times = np.linspace(0.0, T_TOT, N)
    temp = (TEMP_REF + TEMP_RISE * times / (80 * 24 * 365)
            + 10 * np.sin(2 * np.pi / 24 * times)
            + 10 * np.sin(2 * np.pi / (24 * 365) * times))
    I_S = 0.001 + 0.0005 * np.sin(2 * np.pi / (24 * 365) * times)
    I_D = 0.0001 + 5e-05 * np.sin(2 * np.pi / (24 * 365) * times)
    return temp, I_S, I_D


def _precompute(theta):
    """float64 propagator weights, packed into the device operand layouts."""
    theta = np.asarray(theta, np.float64)
    (kSr, kDr, kMr, EaS, EaD, EaM, aSD, aDS, aM, aMSC, uM, cS, cD, cM) = theta
    temp, I_S, I_D = _forcings()
    arr = lambda p, Ea: p * np.exp(-Ea / GAS_R * (1.0 / temp - 1.0 / TEMP_REF))
    k_S, k_D, k_M = arr(kSr, EaS), arr(kDr, EaD), arr(kMr, EaM)

    zeros = np.zeros(N)
    A0 = np.stack([-k_S, aDS * k_D, aM * aMSC * k_M])
    A1 = np.stack([aSD * k_S, -(uM + k_D), aM * (1 - aMSC) * k_M])
    A2 = np.stack([zeros, np.full(N, uM), -k_M])
    W = np.stack([A0, A1, A2]).transpose(2, 0, 1)          # [N,3,3]
    bias = np.stack([I_S, I_D, zeros], axis=1)             # [N,3]

    beta = np.clip(np.array([cS, cD, cM]), 1e-6, None)
    ds = np.sqrt(beta * DT)

    M = np.eye(3)[None] + DT * W[1:]                       # [10000,3,3]
    c = DT * bias[1:]                                      # [10000,3]

    # level A: within-chunk suffix products S10[c,tau] = M_{end}...M_{tau+1}
    Mc = M.reshape(NC1, L1, 3, 3)
    S10 = np.empty((NC1, L1, 3, 3))
    A10 = np.empty((NC1, 3, 3))
    for cI in range(NC1):
        acc = np.eye(3)
        S10[cI, L1 - 1] = acc
        for tau in range(L1 - 2, -1, -1):
            acc = acc @ Mc[cI, tau + 1]
            S10[cI, tau] = acc
        A10[cI] = S10[cI, 0] @ Mc[cI, 0]
    Gmat = (S10 * ds[None, None, None, :]).transpose(0, 1, 3, 2).reshape(NC1, 30, 3)

    # level B: within-window suffix products over chunks
    A10w = A10.reshape(NW, CPW, 3, 3)
    S50 = np.empty((NW, CPW, 3, 3))
    A50 = np.empty((NW, 3, 3))
    for w in range(NW):
        acc = np.eye(3)
        S50[w, CPW - 1] = acc
        for g in range(CPW - 2, -1, -1):
            acc = acc @ A10w[w, g + 1]
            S50[w, g] = acc
        A50[w] = S50[w, 0] @ A10w[w, 0]
    Hmat = S50.transpose(0, 1, 3, 2).reshape(NW, 3 * CPW, 3)   # [w, 3g+j, i]

    # deterministic trajectory at obs points (exact, float64)
    xd = np.zeros(3)
    detx = np.zeros((NOBS, 3))
    for t in range(NSTEP):
        xd = M[t] @ xd + c[t]
        if (t + 1) % OBS_EVERY == 0:
            detx[(t + 1) // OBS_EVERY] = xd

    # observation weights
    sub = np.arange(NOBS) * OBS_EVERY
    C1 = np.stack([(1 - aSD) * k_S[sub], (1 - aDS) * k_D[sub], (1 - aM) * k_M[sub]],
                  axis=1)
    Wobs = np.concatenate([np.broadcast_to(np.eye(3), (NOBS, 3, 3)),
                           C1[:, None, :]], axis=1)        # [NOBS,4,3]

    # level C: Rmat[(w,j),(n,o)] = sum_i Wobs[n,o,i] PhiW[n,w+1][i,j] (w < n)
    Rmat = np.zeros((3 * NW, NOUT))
    RX = np.zeros((3, NOUT))
    base = np.zeros(NOUT)
    for n in range(NOBS):
        WP = Wobs[n]
        base[4 * n:4 * n + 4] = WP @ detx[n]
        acc = WP.copy()
        for w in range(n - 1, -1, -1):
            Rmat[3 * w:3 * w + 3, 4 * n:4 * n + 4] = acc.T
            acc = acc @ A50[w]
        RX[:, 4 * n:4 * n + 4] = acc.T

    # ---------------- pack into device layouts (bf16) ----------------
    # Gdense [128, 750]: chunk block g lives at partitions 32g..32g+29 (rows
    # 30,31 zero pad).  Gdense[32g+r, 3s+i] = Gmat[4s+g, r, i].  The device
    # scatters this into the block-diag lhsT layout with 4 strided DVE copies.
    Gdense = np.zeros((SUPER, 32, NMM_A, 3), BF16)
    Gdense[:, :30] = Gmat.reshape(NMM_A, SUPER, 30, 3).transpose(1, 2, 0, 3)
    Gdense = Gdense.reshape(KE, NMM_A * 3)

    # u10 row map: chunk c10, comp i -> row 32*((c10//4)%4) + 3*(c10%4) + i,
    #                                   col 64*(c10//16) + b
    # HB [128, 80*30]: level-B lhsT tiles; matmul mB = 4*slot + (tau - tau0)
    HB = np.zeros((128, NSLOT_B * TAUS_PER_SLOT * 30), np.float64)
    mB = 0
    for om in range(NSLOT_B):
        tau0 = (50 * om) // 16
        for tau in range(tau0, tau0 + TAUS_PER_SLOT):
            blk = HB[:, 30 * mB:30 * (mB + 1)]
            for rho in range(128):
                q = rho % 32
                if q >= 12:
                    continue
                c10 = 16 * tau + 4 * (rho // 32) + q // 3
                jj = q % 3
                if c10 >= NC1:
                    continue
                w = c10 // 5
                if w // WPS != om:
                    continue
                m = w - WPS * om
                g = c10 - CPW * w
                blk[rho, 3 * m:3 * m + 3] = Hmat[w, 3 * g + jj, :]
            mB += 1
    # slot 19 keeps only its first (tau=59) matmul: windows 190-191 live
    # entirely in tau 59, and windows 192-199 go through the direct path.
    # Only rows with rho%32 < 12 are nonzero: ship those 48 rows; the rest
    # are zeroed on-device (they multiply u10 pad rows, which are zero, but
    # must not be NaN bit patterns).
    HB = HB[:, :30 * 77].astype(BF16)

    # direct-path weights: eps of chunks 960-999 -> output cols [772:804].
    # Wd[32g + 3tau + j, 32c + o] = sum_i Rmat[3w+i, 772+o] *
    #   (S50[w,g'] @ S10[c10,tau])[i,j] * ds[j]   for c10 = 4(240+c)+g
    Wd = np.zeros((128, NSD * NOD), np.float64)
    for c in range(NSD):
        for g in range(SUPER):
            c10 = 4 * (240 + c) + g
            w, gg = c10 // CPW, c10 % CPW
            R3 = Rmat[3 * w:3 * w + 3, NOUT - NOD:]          # [3, 32]
            for tau in range(L1):
                M1 = S50[w, gg] @ S10[c10, tau]               # [3,3]
                blk = (R3.T @ M1) * ds[None, :]               # [32, 3]
                for j in range(3):
                    Wd[32 * g + 3 * tau + j, NOD * c:NOD * (c + 1)] = blk[:, j]
    Wd = Wd.astype(BF16)

    # u50 row map: window w, comp j -> row 32*((w//10)%4) + 3*(w%10) + j,
    #                                  col 64*(w//40) + b
    # Rsb: only the nonzero (triangular) column range of each (wt, half)
    # block is shipped; see _rsb_blocks() for the packing.
    blocks = _rsb_blocks()
    ncols = sum(b[3] for b in blocks)
    Rsb = np.zeros((128, ncols), BF16)
    for wt, h, rel0, keep, off in blocks:
        blk = np.zeros((128, keep), np.float64)
        for rho in range(128):
            q = rho % 32
            if q >= 30:
                continue
            w = WPS * (4 * wt + rho // 32) + q // 3
            j = q % 3
            blk[rho] = Rmat[3 * w + j, NH * h + rel0:NH * h + rel0 + keep]
        Rsb[:, off:off + keep] = blk

    return dict(Gdense=Gdense, HB=HB, Rsb=Rsb, Wd=Wd,
                RX=np.asarray(RX), base=np.asarray(base))


def _rsb_blocks():
    """Nonzero column ranges of each level-C (wt, half) block.

    Window-tile wt covers windows [40wt, 40wt+40); its rows only affect
    observations n >= 40wt+1, i.e. global cols >= 4*(40wt+1).  Returns
    (wt, h, rel0, keep, packed_col_offset) for each nonempty block.
    """
    blocks = []
    off = 0
    for h in range(2):
        for wt in range(NTILE_B):
            rel0 = max(0, 4 * (40 * wt + 1) - NH * h)
            if rel0 >= NH:
                continue
            keep = NH - rel0
            blocks.append((wt, h, rel0, keep, off))
            off += keep
    return blocks


def _pack_eps(noise_core):
    """[64,10000,3] f32 -> bf16 [128, 250*64]: row 32g + (3tau+j), col
    64s + b = eps[b, t, j] for t = 10*(4s+g) + tau; rows 32g+30, 32g+31 are
    zero pad."""
    a = noise_core.reshape(BC, NSTEP * 3).T          # [30000, 64] view
    a = np.ascontiguousarray(a).reshape(NSUP_COL, SUPER, 30, BC)
    out = np.zeros((SUPER, 32, NSUP_COL, BC), BF16)
    out[:, :30] = a.transpose(1, 2, 0, 3)
    return out.reshape(KE, NSUP_COL * BC)


# ------------------------------------------------------------ bass program
def _build_program(**bass_kwargs):
    import concourse.bass as bass
    import concourse.tile as tile
    from concourse import bacc, mybir

    f32 = mybir.dt.float32
    bf16 = mybir.dt.bfloat16
    nc = bacc.Bacc(None, target_bir_lowering=False, **bass_kwargs)

    eps_d = nc.dram_tensor("eps", [KE, NSUP_COL * BC], bf16, kind="ExternalInput")
    gsb_d = nc.dram_tensor("gsb", [KE, NMM_A * 3], bf16, kind="ExternalInput")
    hb_d = nc.dram_tensor("hb", [128, 30 * 77], bf16, kind="ExternalInput")
    wd_d = nc.dram_tensor("wd", [128, NSD * NOD], bf16, kind="ExternalInput")
    rblocks = _rsb_blocks()
    NRSB = sum(b[3] for b in rblocks)
    rsb_d = nc.dram_tensor("rsb", [128, NRSB], bf16, kind="ExternalInput")
    aff_d = nc.dram_tensor("aff", [BC, NOUT], bf16, kind="ExternalInput")
    out_d = nc.dram_tensor("out", [BC, NOUT], bf16, kind="ExternalOutput")

    with tile.TileContext(nc) as tc:
        with (
            tc.tile_pool(name="consts", bufs=1) as consts,
            tc.tile_pool(name="epsp", bufs=1) as epsp,
            tc.tile_pool(name="psA", bufs=1, space="PSUM") as psA,
            tc.tile_pool(name="psB", bufs=1, space="PSUM") as psB,
            tc.tile_pool(name="psC", bufs=1, space="PSUM") as psC,
        ):
            gdt = consts.tile([KE, NMM_A * 3], bf16)
            gsb = consts.tile([KE, NMM_A * 12 + 12], bf16)  # 12 pad cols for the
            hb = consts.tile([128, 30 * 77], bf16)          # AP split
            wdt = consts.tile([128, NSD * NOD], bf16)
            rsb = consts.tile([128, NRSB], bf16)
            afft = consts.tile([BC, NOUT], bf16)
            u10 = consts.tile([128, NTILE_A * BC], bf16)
            u50 = consts.tile([128, NTILE_B * BC], bf16)
            outsb = consts.tile([BC, NOUT], bf16)

            eps_t = []
            for i, nsup in enumerate(SLICES):
                et = epsp.tile([KE, BC * nsup], bf16, tag=f"eps{i}")
                eps_t.append(et)

            zt = consts.tile([128, 512], bf16)
            # PSUM zeroing is done by the PE itself (zero-weight matmuls)
            # during the dead startup window, so the DVE prologue stays
            # short; only the tiny zt memset is needed first.
            nc.vector.memset(zt.bitcast(f32), 0.0)

            def dma_eps(i):
                nc.sync.dma_start(out=eps_t[i],
                                  in_=eps_d[:, BC * SOFF[i]:BC * SOFF[i + 1]])

            # DMA issue order = HWDGE FIFO order per ring.  The load is
            # balanced across BOTH HWDGE rings (sync + scalar, ~2.7 MB each)
            # so neither ring idles: total stream time is HBM-limited rather
            # than single-ring-limited, and per-transfer dead time overlaps.
            nc.sync.dma_start(out=gdt, in_=gsb_d[:])
            dma_eps(0)
            nc.sync.dma_start(out=hb, in_=hb_d[:])

            # build the block-diag level-A lhsT from the dense G on-device:
            # gsb[32g+r, 12s+3g+i] = gdt[32g+r, 3s+i].  The memset runs
            # through an f32 view (bf16 memset is ~2.7x slower per byte).
            nc.vector.memset(gsb.bitcast(f32), 0.0)
            for g in range(SUPER):
                nc.vector.tensor_copy(
                    gsb[32 * g:32 * (g + 1), 3 * g:3 * g + 12 * NMM_A]
                    .rearrange("p (s c) -> p s c", c=12)[:, :, 0:3],
                    gdt[32 * g:32 * (g + 1), :]
                    .rearrange("p (s c) -> p s c", c=3))
            dma_eps(1)
            dma_eps(2)
            dma_eps(3)
            dma_eps(4)
            nc.sync.dma_start(out=afft, in_=aff_d[:])
            nc.sync.dma_start(out=rsb, in_=rsb_d[:])
            nc.sync.dma_start(out=wdt, in_=wd_d[:])
            dma_eps(5)

            def eps_rhs(s):
                for i in range(len(SLICES)):
                    if s < SOFF[i + 1]:
                        c = s - SOFF[i]
                        return eps_t[i][:, BC * c:BC * (c + 1)]
                raise AssertionError(s)

            # ---- level A: 250 matmuls -> u10 ----
            # 3 persistent PSUM tiles used round-robin; the PE zeroes them
            # itself before any data arrives (pad rows must hold real floats,
            # not NaN bit patterns, since 0*NaN = NaN)
            pa_t = []
            for i in range(3):
                pa_i = psA.tile([128, BC * 8], f32, tag=f"pa{i}")
                pa_t.append(pa_i)
            for pa in pa_t:
                nc.tensor.matmul(pa, zt[:, 0:128], zt[:, 0:BC * 8],
                                 start=True, stop=True)

            # u10 casts alternate DVE / ACT so evacuation never serializes
            # behind a single engine
            def emit_bigA(big):
                sg_lo, sg_hi = BIGS[big]
                ncol = BC * (sg_hi - sg_lo)
                pa = pa_t[big % 3]
                for sgA in range(sg_lo, sg_hi):
                    co = BC * (sgA - sg_lo)
                    nmm = 4 if sgA < NTILE_A - 1 else NMM_A - 4 * (NTILE_A - 1)
                    for sig in range(nmm):
                        s = 4 * sgA + sig
                        nc.tensor.matmul(
                            pa[32 * sig:32 * sig + 12, co:co + BC],
                            gsb[:, 12 * s:12 * (s + 1)],
                            eps_rhs(s),
                            start=True, stop=True, tile_position=(0, 32 * sig),
                            # sim's group checker mis-maps offset outs
                            skip_group_check=(sig != 0 or sgA != sg_lo))
                eng = nc.vector.tensor_copy if big % 2 == 0 else nc.scalar.copy
                eng(u10[:, BC * sg_lo:BC * sg_lo + ncol], pa[:, :ncol])

            # ---- level B: 80 matmuls -> u50 (one PSUM bank for all 5 tiles) --
            pb = psB.tile([128, BC * NTILE_B], f32, tag="pb")
            nc.tensor.matmul(pb, zt[:, 0:128], zt[:, 0:BC * NTILE_B],
                             start=True, stop=True)

            def emit_slotB(om):
                wt, sb = om // 4, om % 4
                tau0 = (50 * om) // 16
                # slot 19's windows (190, 191) live entirely in tau 59;
                # windows 192-199 go through the direct path instead
                nti = 1 if om == 19 else TAUS_PER_SLOT
                for ti in range(nti):
                    tau = tau0 + ti
                    mB = TAUS_PER_SLOT * om + ti
                    nc.tensor.matmul(
                        pb[32 * sb:32 * sb + 30, BC * wt:BC * (wt + 1)],
                        hb[:, 30 * mB:30 * (mB + 1)],
                        u10[:, BC * tau:BC * (tau + 1)],
                        start=(ti == 0), stop=(ti == nti - 1),
                        tile_position=(0, 32 * sb),
                        skip_group_check=(om != 0))

            # ---- level C ----
            # noise-propagation matmuls accumulate into pc; the affine part
            # (afft) is added during evacuation.  h=1 is split so only the
            # wt=4 block (output cols [242:402] of the half) sits behind the
            # final eps slice.
            CB = {(b[0], b[1]): b for b in rblocks}

            def mmC(pc, wt, h, start, stop, skip=False):
                _, _, rel0, keep, off = CB[(wt, h)]
                nc.tensor.matmul(pc[:, rel0:rel0 + keep],
                                 u50[:, BC * wt:BC * (wt + 1)],
                                 rsb[:, off:off + keep],
                                 start=start, stop=stop, skip_group_check=skip)

            # ---- emission order == PE execution order: interleave level-A
            # bigs, level-B slots, and level-C pieces by data arrival so the
            # PE FIFO never head-of-line blocks on a later eps slice ----
            emit_bigA(0)
            emit_bigA(1)
            for om in range(5):            # taus <= 15 (big 1) + hb
                emit_slotB(om)
            emit_bigA(2)
            emit_bigA(3)
            for om in range(5, 10):        # taus <= 31 (big 3)
                emit_slotB(om)
            emit_bigA(4)
            emit_bigA(5)
            for om in range(10, 15):       # taus <= 46 (big 5)
                emit_slotB(om)
            nc.scalar.copy(u50[:, :BC * 3], pb[:, :BC * 3])
            emit_bigA(6)
            emit_bigA(7)
            for om in (15, 16):            # taus <= 53 (big 6)
                emit_slotB(om)
            nc.scalar.copy(u50[:, BC * 3:BC * 4], pb[:, BC * 3:BC * 4])
            for om in (17, 18, 19):        # taus <= 59 (big 7)
                emit_slotB(om)
            nc.scalar.copy(u50[:, BC * 4:BC * 5], pb[:, BC * 4:BC * 5])

            # ---- level C: all after the stream-gated work (rsb lands
            # mid-stream on the scalar ring; putting these earlier would
            # head-of-line block the PE FIFO on it) ----
            pc0 = psC.tile([BC, NH], f32, tag="pc0")
            for bi, wt in enumerate([0, 1, 2]):
                mmC(pc0, wt, 0, start=(bi == 0), stop=(wt == 2))
            nc.vector.tensor_add(outsb[:, 4:NH], pc0[:, 4:], afft[:, 4:NH])
            nc.vector.tensor_copy(outsb[:, 0:4], afft[:, 0:4])
            nc.sync.dma_start(out=out_d[:, 0:NH], in_=outsb[:, 0:NH])

            pc1 = psC.tile([BC, NH], f32, tag="pc1")
            for bi, wt in enumerate([0, 1, 2]):
                mmC(pc1, wt, 1, start=(bi == 0), stop=False)
            mmC(pc1, 3, 1, start=False, stop=True)   # closes cols [0:242]
            nc.vector.tensor_add(outsb[:, NH:NH + 242], pc1[:, 0:242],
                                 afft[:, NH:NH + 242])
            nc.sync.dma_start(out=out_d[:, NH:NH + 242],
                              in_=outsb[:, NH:NH + 242])
            # wt4's windows >= 192 rows of u50 are zero; the direct path
            # covers them by accumulating onto pc1's preserved has_written
            mmC(pc1, 4, 1, start=False, stop=False, skip=True)

            # ---- the tail: the final 154 KB eps slice feeds 10 direct
            # matmuls (eps stationary, wd moving) straight into pc1 ----
            for c in range(NSD):
                nc.tensor.matmul(
                    pc1[:, NH - NOD:], eps_t[5][:, BC * c:BC * (c + 1)],
                    wdt[:, NOD * c:NOD * (c + 1)],
                    start=False, stop=(c == NSD - 1), skip_group_check=True)
            nc.vector.tensor_add(outsb[:, NH + 242:], pc1[:, 242:],
                                 afft[:, NH + 242:])
            nc.sync.dma_start(out=out_d[:, NH + 242:],
                              in_=outsb[:, NH + 242:])

    nc.finalize()
    return nc


# ------------------------------------------------------------------ kernel
def kernel(theta, x0, noise, obs_every):
    global _program_cache, _last_results
    from concourse.bass_utils import run_bass_kernel_spmd

    assert int(obs_every) == OBS_EVERY
    theta = np.asarray(theta, np.float32)
    x0 = np.asarray(x0, np.float32)
    noise = np.asarray(noise, np.float32)

    ops = _precompute(theta.astype(np.float64))

    if _program_cache is None:
        _program_cache = _build_program()
    nc = _program_cache

    in_maps = []
    for q in range(NCORE):
        sl = slice(BC * q, BC * (q + 1))
        aff = (x0[sl].astype(np.float64) @ ops["RX"]
               + ops["base"][None]).astype(BF16)
        in_maps.append({
            "eps": _pack_eps(noise[sl]),
            "gsb": ops["Gdense"],
            "hb": ops["HB"],
            "rsb": ops["Rsb"],
            "wd": ops["Wd"],
            "aff": aff,
        })

    import os
    trace = bool(os.environ.get("KERNEL_TRACE"))
    res = run_bass_kernel_spmd(nc, in_maps, core_ids=list(range(NCORE)),
                               trace=trace)
    _last_results = res
    out = np.concatenate(
        [res.results[q]["out"].reshape(BC, NOBS, 4) for q in range(NCORE)],
        axis=0)
    return out.astype(np.float32)
